# revision 8
# baseline (speedup 1.0000x reference)
"""Bass/Trainium2 kernel for nn_BehaviorSpecificPFF (MoE-style routed FFN).

Reference semantics (per token t):
    e = b_seq[t]
    out[t] = 0                                   if e == 0
    out[t] = relu(x[t] @ W1[e-1] + b1[e-1]) @ W2[e-1] + b2[e-1]   otherwise

Strategy:
  - Data parallel over batch: 32 batches -> 4 per core on 8 cores.
  - Per core (8192 tokens), entirely on device:
      1. Routing scan: from b_seq compute, for every token, a unique slot in a
         per-expert bucket (matmul-based cross-partition prefix sum + shifted-add
         in-row prefix sum). Scatter token ids into two DRAM index arrays
         (gather-index, init 0; scatter-index, init BIG so padding slots are
         dropped by the bounds check).
      2. For each expert bucket, in supertiles of up to 512 slots: indirect-DMA
         gather x rows, PE-transpose to [d, tok], two matmul layers (fp32 data,
         fp32r matmul mode) with bias+relu fused on the ACT engine, PE-transpose
         back to [tok, d], indirect-DMA scatter rows to the output (padding slots
         dropped via bounds check; expert-0 rows stay zero from the zero-init
         output buffer).
  - Bucket capacities are specialized per call (max over cores, rounded to 128);
    the kernel is otherwise input-agnostic.
"""

import numpy as np

import concourse.bass as bass
import concourse.tile as tile
from concourse import bacc, mybir
from concourse.bass import IndirectOffsetOnAxis
from concourse.bass_utils import run_bass_kernel_spmd
from concourse.masks import make_identity

N_CORES = 8
B, T, D, DFF, NB = 32, 2048, 256, 1024, 4
P = 128
NTOK = B * T // N_CORES          # 8192 tokens per core
JCOL = NTOK // P                 # 64 scan columns
BIG = 100000
F32 = mybir.dt.float32
F32R = mybir.dt.float32r
I32 = mybir.dt.int32
AF = mybir.ActivationFunctionType
ALU = mybir.AluOpType


def build_nc(caps, mm_dtype=F32R, debug=False):
    """Build the per-core Bass program. caps: slot capacity per expert (mult of 128)."""
    ntiles = [c // P for c in caps]
    nslot = sum(caps)
    ntt = nslot // P                       # total 128-slot tiles
    bases = [sum(caps[:e]) for e in range(NB)]

    nc = bacc.Bacc("TRN2", target_bir_lowering=False, debug=False,
                   num_devices=N_CORES)
    x_d = nc.dram_tensor("x", [NTOK, D], F32, kind="ExternalInput").ap()
    b_d = nc.dram_tensor("b", [NTOK], I32, kind="ExternalInput").ap()
    w1_d = nc.dram_tensor("w1s", [P, 2 * NB * DFF], F32R, kind="ExternalInput").ap()
    w2_d = nc.dram_tensor("w2s", [P, (DFF // P) * NB * D], F32R, kind="ExternalInput").ap()
    b1_d = nc.dram_tensor("b1s", [P, NB * (DFF // P)], F32, kind="ExternalInput").ap()
    b2_d = nc.dram_tensor("b2s", [P, NB * (D // P)], F32, kind="ExternalInput").ap()
    y_d = nc.dram_tensor("y", [NTOK, D], F32, kind="ExternalOutput").ap()
    skind = "ExternalOutput" if debug else "Internal"
    dbg = {}
    if debug:
        for nm, cols in [("b_f", JCOL), ("M", NB * JCOL), ("incl", NB * JCOL),
                         ("cnt", NB), ("exr", NB), ("cand", NB * JCOL),
                         ("perm_f", JCOL)]:
            dbg[nm] = nc.dram_tensor("dbg_" + nm, [P, cols], F32, kind="ExternalOutput").ap()
        for nm, cols in [("perm_i", JCOL), ("tid", JCOL)]:
            dbg[nm] = nc.dram_tensor("dbg_" + nm, [P, cols], I32, kind="ExternalOutput").ap()
    sarr = nc.dram_tensor("sarr", [nslot, 1], I32, kind=skind).ap()

    with tile.TileContext(nc) as tc:
        _body(tc, x_d, b_d, w1_d, w2_d, b1_d, b2_d, y_d, sarr,
              caps, ntiles, bases, nslot, ntt, mm_dtype, dbg)
    nc.compile()
    return nc


def _body(tc, x_d, b_d, w1_d, w2_d, b1_d, b2_d, y_d, sarr,
          caps, ntiles, bases, nslot, ntt, mm_dtype, dbg=None):
    nc = tc.nc
    nv = nc.vector
    ns = nc.scalar
    ng = nc.gpsimd
    sy = nc.sync

    import contextlib
    ctx = contextlib.ExitStack()
    with ctx:
        const = ctx.enter_context(tc.tile_pool(name="const", bufs=1))
        scan = ctx.enter_context(tc.tile_pool(name="scan", bufs=1))
        idxp = ctx.enter_context(tc.tile_pool(name="idx", bufs=3))
        xgp = ctx.enter_context(tc.tile_pool(name="xg", bufs=3))
        xtp = ctx.enter_context(tc.tile_pool(name="xt", bufs=4))
        htp = ctx.enter_context(tc.tile_pool(name="ht", bufs=10))
        ytp = ctx.enter_context(tc.tile_pool(name="yt", bufs=4))
        yop = ctx.enter_context(tc.tile_pool(name="yo", bufs=3))
        ps_int = ctx.enter_context(tc.tile_pool(name="ps_int", bufs=2, space="PSUM"))
        ps_h = ctx.enter_context(tc.tile_pool(name="ps_h", bufs=2, space="PSUM"))
        ps_y = ctx.enter_context(tc.tile_pool(name="ps_y", bufs=2, space="PSUM"))
        ps_outt = ctx.enter_context(tc.tile_pool(name="ps_outt", bufs=2, space="PSUM"))

        # ---- constants / weights -------------------------------------------
        ident = const.tile([P, P], F32)
        make_identity(nc, ident[:])
        ltri = const.tile([P, P], F32)                 # ltri[k, m] = 1 if k < m
        ng.memset(ltri[:], 1.0)
        ng.affine_select(out=ltri[:], in_=ltri[:], compare_op=ALU.is_gt,
                         fill=0.0, base=0, pattern=[[1, P]], channel_multiplier=-1)

        w1s = const.tile([P, 2 * NB * DFF], F32R)
        sy.dma_start(w1s[:], w1_d[:])
        w2s = const.tile([P, (DFF // P) * NB * D], F32R)
        sy.dma_start(w2s[:], w2_d[:])
        b1s = const.tile([P, NB * (DFF // P)], F32)
        sy.dma_start(b1s[:], b1_d[:])
        b2s = const.tile([P, NB * (D // P)], F32)
        sy.dma_start(b2s[:], b2_d[:])

        # ---- init index arrays in DRAM -------------------------------------
        bt = const.tile([P, ntt], I32)
        ng.memset(bt[:], BIG)
        sarr_cov = sarr.rearrange("(p t) o -> p (t o)", p=P)
        sy.dma_start(sarr_cov[:, :], bt[:])

        # ---- phase 1: routing scan -----------------------------------------
        b_i = scan.tile([P, JCOL], I32)
        sy.dma_start(b_i[:], b_d.rearrange("(p j) -> p j", p=P))
        b_f = scan.tile([P, JCOL], F32)
        nv.tensor_copy(b_f[:], b_i[:])

        # masks per expert: M[p, e, j] = (b == e+1)
        M = scan.tile([P, NB * JCOL], F32)
        M3 = M[:].rearrange("p (e j) -> p e j", e=NB)
        for e in range(NB):
            nv.tensor_scalar(M3[:, e, :], b_f[:], float(e + 1), None, ALU.is_equal)

        # in-row inclusive prefix sum along j (Hillis-Steele, ping-pong)
        sA = scan.tile([P, NB * JCOL], F32)
        sB = scan.tile([P, NB * JCOL], F32)
        cur, nxt = M, sA
        s = 1
        while s < JCOL:
            c3 = cur[:].rearrange("p (e j) -> p e j", e=NB)
            n3 = nxt[:].rearrange("p (e j) -> p e j", e=NB)
            nv.tensor_copy(n3[:, :, 0:s], c3[:, :, 0:s])
            nv.tensor_add(n3[:, :, s:JCOL], c3[:, :, s:JCOL], c3[:, :, 0:JCOL - s])
            cur = nxt
            nxt = sB if cur is sA else sA
            s *= 2
        incl = cur                                        # [P, NB*JCOL]

        # per-row counts and cross-partition exclusive prefix (via matmul)
        cnt = scan.tile([P, NB], F32)
        nv.tensor_reduce(cnt[:], M3[:, :, :], mybir.AxisListType.X, ALU.add)
        exr_ps = ps_h.tile([P, NB], F32, tag="hps", name="exr_ps")
        nc.tensor.matmul(exr_ps[:], ltri[:], cnt[:], start=True, stop=True)
        exr = scan.tile([P, NB], F32)
        nv.tensor_copy(exr[:], exr_ps[:])

        # candidate slot per (token, expert); select by mask; BIG for expert 0
        cand = scan.tile([P, NB * JCOL], F32)
        c3 = cand[:].rearrange("p (e j) -> p e j", e=NB)
        i3 = incl[:].rearrange("p (e j) -> p e j", e=NB)
        for e in range(NB):
            nv.tensor_scalar(c3[:, e, :], i3[:, e, :], exr[:, e:e + 1],
                             float(bases[e] - 1), ALU.add, ALU.add)
        prod = scan.tile([P, NB * JCOL], F32)
        nv.tensor_tensor(out=prod[:], in0=M[:], in1=cand[:], op=ALU.mult)
        perm_f = scan.tile([P, JCOL], F32)
        nv.tensor_reduce(perm_f[:],
                         prod[:].rearrange("p (e j) -> p j e", e=NB),
                         mybir.AxisListType.X, ALU.add)
        m0s = scan.tile([P, JCOL], F32)
        nv.tensor_scalar(m0s[:], b_f[:], 0.0, float(BIG), ALU.is_equal, ALU.mult)
        nv.tensor_add(perm_f[:], perm_f[:], m0s[:])
        perm_i = scan.tile([P, JCOL], I32)
        nv.tensor_copy(perm_i[:], perm_f[:])

        tid = scan.tile([P, JCOL], I32)
        ng.iota(tid[:], pattern=[[1, JCOL]], base=0, channel_multiplier=JCOL)
        if dbg:
            for nm, t in [("b_f", b_f), ("M", M), ("incl", incl), ("cnt", cnt),
                          ("exr", exr), ("cand", cand), ("perm_f", perm_f),
                          ("perm_i", perm_i), ("tid", tid)]:
                sy.dma_start(dbg[nm][:, :], t[:])

        for j in range(JCOL):
            ng.indirect_dma_start(
                out=sarr[:], out_offset=IndirectOffsetOnAxis(ap=perm_i[:, j:j + 1], axis=0),
                in_=tid[:, j:j + 1], in_offset=None,
                bounds_check=nslot - 1, oob_is_err=False)

        sarr_t = sarr.rearrange("(t p) o -> p (t o)", p=P)   # [P, ntt] slot t*128+p

        # ---- phase 2: per-expert FFN over supertiles -----------------------
        for e in range(NB):
            t0_e = bases[e] // P
            nt_e = ntiles[e]
            g0 = 0
            while g0 < nt_e:
                G = min(4, nt_e - g0)
                ntoks = G * P
                t0 = t0_e + g0

                idst = idxp.tile([P, 4], I32, tag="idst")
                sy.dma_start(idst[:, :G], sarr_t[:, t0:t0 + G])
                isrc = idxp.tile([P, 4], I32, tag="isrc")
                nv.tensor_scalar(isrc[:, :G], idst[:, :G], NTOK - 1, None, ALU.min)

                xg = xgp.tile([P, 4 * D], F32)
                for gi in range(G):
                    ng.indirect_dma_start(
                        out=xg[:, gi * D:(gi + 1) * D], out_offset=None,
                        in_=x_d[:], in_offset=IndirectOffsetOnAxis(ap=isrc[:, gi:gi + 1], axis=0))

                # transpose gathered [tok, d] -> xt[k][d_chunk, tok]
                xt = [xtp.tile([P, 512], F32R, tag=f"xt{k}", name=f"xt{k}") for k in range(2)]
                for k in range(2):
                    pst = ps_int.tile([P, 512], F32)
                    for gi in range(G):
                        nc.tensor.transpose(
                            out=pst[:, gi * P:(gi + 1) * P],
                            in_=xg[:, gi * D + k * P: gi * D + (k + 1) * P],
                            identity=ident[:])
                    nv.tensor_copy(xt[k][:, :ntoks], pst[:, :ntoks])

                # layer 1 + fused bias/relu -> ht[m][dff_chunk, tok]
                ht = [htp.tile([P, 512], F32R, tag="ht", name="ht") for _ in range(DFF // P)]
                for m in range(DFF // P):
                    hps = ps_h.tile([P, 512], F32)
                    for k in range(2):
                        nc.tensor.matmul(
                            hps[:, :ntoks],
                            w1s[:, (e * 2 + k) * DFF + m * P:(e * 2 + k) * DFF + (m + 1) * P],
                            xt[k][:, :ntoks],
                            start=(k == 0), stop=(k == 1))
                    ns.activation(ht[m][:, :ntoks], hps[:, :ntoks], AF.Relu,
                                  bias=b1s[:, e * (DFF // P) + m:e * (DFF // P) + m + 1],
                                  scale=1.0)

                # layer 2 + bias -> yt[c][dmodel_chunk, tok]
                yt = [ytp.tile([P, 512], F32, tag="yt", name="yt") for _ in range(D // P)]
                for c in range(D // P):
                    yps = ps_y.tile([P, 512], F32)
                    for k in range(DFF // P):
                        nc.tensor.matmul(
                            yps[:, :ntoks],
                            w2s[:, (e * (DFF // P) + k) * D + c * P:(e * (DFF // P) + k) * D + (c + 1) * P],
                            ht[k][:, :ntoks],
                            start=(k == 0), stop=(k == DFF // P - 1))
                    nv.tensor_scalar(yt[c][:, :ntoks], yps[:, :ntoks],
                                     b2s[:, e * (D // P) + c:e * (D // P) + c + 1],
                                     None, ALU.add)

                # transpose back [dmodel, tok] -> yo[tok, dmodel], 2 blocks/pack
                yo = yop.tile([P, 4 * D], F32)
                for pk in range((G + 1) // 2):
                    gis = [gi for gi in (2 * pk, 2 * pk + 1) if gi < G]
                    pso = ps_outt.tile([P, 512], F32)
                    for bi, gi in enumerate(gis):
                        for c in range(D // P):
                            nc.tensor.transpose(
                                out=pso[:, bi * D + c * P: bi * D + (c + 1) * P],
                                in_=yt[c][:, gi * P:(gi + 1) * P],
                                identity=ident[:])
                    nv.tensor_copy(yo[:, 2 * pk * D: (2 * pk + len(gis)) * D],
                                   pso[:, :len(gis) * D])

                for gi in range(G):
                    ng.indirect_dma_start(
                        out=y_d[:], out_offset=IndirectOffsetOnAxis(ap=idst[:, gi:gi + 1], axis=0),
                        in_=yo[:, gi * D:(gi + 1) * D], in_offset=None,
                        bounds_check=NTOK - 1, oob_is_err=False)
                g0 += G


def prep_inputs(x, W1, b1, W2, b2, b_seq):
    """Shard + pre-layout host-side. Returns (in_maps, caps)."""
    x = np.ascontiguousarray(np.asarray(x, dtype=np.float32))
    W1 = np.asarray(W1, dtype=np.float32)
    b1 = np.asarray(b1, dtype=np.float32)
    W2 = np.asarray(W2, dtype=np.float32)
    b2 = np.asarray(b2, dtype=np.float32)
    b_seq = np.ascontiguousarray(np.asarray(b_seq, dtype=np.int32))

    w1s = np.ascontiguousarray(
        W1.reshape(NB, 2, P, DFF).transpose(2, 0, 1, 3).reshape(P, 2 * NB * DFF))
    w2s = np.ascontiguousarray(
        W2.reshape(NB, DFF // P, P, D).transpose(2, 0, 1, 3).reshape(P, -1))
    b1s = np.ascontiguousarray(
        b1.reshape(NB, DFF // P, P).transpose(2, 0, 1).reshape(P, -1))
    b2s = np.ascontiguousarray(
        b2.reshape(NB, D // P, P).transpose(2, 0, 1).reshape(P, -1))

    bpc = B // N_CORES
    in_maps = []
    counts = np.zeros((N_CORES, NB), dtype=np.int64)
    for c in range(N_CORES):
        xc = x[c * bpc:(c + 1) * bpc].reshape(NTOK, D)
        bc = b_seq[c * bpc:(c + 1) * bpc].reshape(NTOK)
        for e in range(NB):
            counts[c, e] = int((bc == e + 1).sum())
        in_maps.append({"x": np.ascontiguousarray(xc),
                        "b": np.ascontiguousarray(bc),
                        "w1s": w1s, "w2s": w2s, "b1s": b1s, "b2s": b2s})
    caps = [max(P, int(np.ceil(counts[:, e].max() / P)) * P) for e in range(NB)]
    return in_maps, caps


def assemble(results):
    bpc = B // N_CORES
    out = np.empty((B, T, D), dtype=np.float32)
    for c in range(N_CORES):
        out[c * bpc:(c + 1) * bpc] = results[c]["y"].reshape(bpc, T, D)
    return out


def kernel(x, W1, b1, W2, b2, b_seq):
    in_maps, caps = prep_inputs(x, W1, b1, W2, b2, b_seq)
    nc = build_nc(caps)
    res = run_bass_kernel_spmd(nc, in_maps, core_ids=list(range(N_CORES)))
    return assemble(res.results)


# revision 9
# speedup vs baseline: 7.2971x; 7.2971x over previous
"""Bass/Trainium2 kernel for nn_BehaviorSpecificPFF (MoE-style routed FFN).

Reference semantics (per token t):
    e = b_seq[t]
    out[t] = 0                                   if e == 0
    out[t] = relu(x[t] @ W1[e-1] + b1[e-1]) @ W2[e-1] + b2[e-1]   otherwise

Strategy:
  - Data parallel over batch: 32 batches -> 4 per core on 8 cores.
  - Per core (8192 tokens), entirely on device:
      1. Routing scan: from b_seq compute, for every token, a unique slot in a
         per-expert bucket (matmul-based cross-partition prefix sum + shifted-add
         in-row prefix sum). Scatter token ids into two DRAM index arrays
         (gather-index, init 0; scatter-index, init BIG so padding slots are
         dropped by the bounds check).
      2. For each expert bucket, in supertiles of up to 512 slots: indirect-DMA
         gather x rows, PE-transpose to [d, tok], two matmul layers (fp32 data,
         fp32r matmul mode) with bias+relu fused on the ACT engine, PE-transpose
         back to [tok, d], indirect-DMA scatter rows to the output (padding slots
         dropped via bounds check; expert-0 rows stay zero from the zero-init
         output buffer).
  - Bucket capacities are specialized per call (max over cores, rounded to 128);
    the kernel is otherwise input-agnostic.
"""

import numpy as np

import concourse.bass as bass
import concourse.tile as tile
from concourse import bacc, mybir
from concourse.bass import IndirectOffsetOnAxis
from concourse.bass_utils import run_bass_kernel_spmd
from concourse.masks import make_identity

N_CORES = 8
B, T, D, DFF, NB = 32, 2048, 256, 1024, 4
P = 128
NTOK = B * T // N_CORES          # 8192 tokens per core
JCOL = NTOK // P                 # 64 scan columns
BIG = 100000
F32 = mybir.dt.float32
F32R = mybir.dt.float32r
I32 = mybir.dt.int32
AF = mybir.ActivationFunctionType
ALU = mybir.AluOpType


def build_nc(caps, mm_dtype=F32R, debug=False, reps=1):
    """Build the per-core Bass program. caps: slot capacity per expert (mult of 128)."""
    ntiles = [c // P for c in caps]
    nslot = sum(caps)
    ntt = nslot // P                       # total 128-slot tiles
    bases = [sum(caps[:e]) for e in range(NB)]

    nc = bacc.Bacc("TRN2", target_bir_lowering=False, debug=False,
                   num_devices=N_CORES)
    x_d = nc.dram_tensor("x", [NTOK, D], F32, kind="ExternalInput").ap()
    b_d = nc.dram_tensor("b", [NTOK], I32, kind="ExternalInput").ap()
    w1_d = nc.dram_tensor("w1s", [P, 2 * NB * DFF], F32R, kind="ExternalInput").ap()
    w2_d = nc.dram_tensor("w2s", [P, (DFF // P) * NB * D], F32R, kind="ExternalInput").ap()
    b1_d = nc.dram_tensor("b1s", [P, NB * (DFF // P)], F32, kind="ExternalInput").ap()
    b2_d = nc.dram_tensor("b2s", [P, NB * (D // P)], F32, kind="ExternalInput").ap()
    y_d = nc.dram_tensor("y", [NTOK, D], F32, kind="ExternalOutput").ap()
    skind = "ExternalOutput" if debug else "Internal"
    dbg = {}
    if debug:
        for nm, cols in [("b_f", JCOL), ("M", NB * JCOL), ("incl", NB * JCOL),
                         ("cnt", NB), ("exr", NB), ("cand", NB * JCOL),
                         ("perm_f", JCOL)]:
            dbg[nm] = nc.dram_tensor("dbg_" + nm, [P, cols], F32, kind="ExternalOutput").ap()
        for nm, cols in [("perm_i", JCOL), ("tid", JCOL)]:
            dbg[nm] = nc.dram_tensor("dbg_" + nm, [P, cols], I32, kind="ExternalOutput").ap()
    sarr = nc.dram_tensor("sarr", [nslot, 1], I32, kind=skind).ap()

    with tile.TileContext(nc) as tc:
        _body(tc, x_d, b_d, w1_d, w2_d, b1_d, b2_d, y_d, sarr,
              caps, ntiles, bases, nslot, ntt, mm_dtype, dbg, reps)
    nc.compile()
    return nc


def _body(tc, x_d, b_d, w1_d, w2_d, b1_d, b2_d, y_d, sarr,
          caps, ntiles, bases, nslot, ntt, mm_dtype, dbg=None, reps=1):
    nc = tc.nc
    nv = nc.vector
    ns = nc.scalar
    ng = nc.gpsimd
    sy = nc.sync

    import contextlib
    ctx = contextlib.ExitStack()
    with ctx:
        const = ctx.enter_context(tc.tile_pool(name="const", bufs=1))
        scan = ctx.enter_context(tc.tile_pool(name="scan", bufs=1))
        idxp = ctx.enter_context(tc.tile_pool(name="idx", bufs=3))
        xgp = ctx.enter_context(tc.tile_pool(name="xg", bufs=3))
        xtp = ctx.enter_context(tc.tile_pool(name="xt", bufs=4))
        htp = ctx.enter_context(tc.tile_pool(name="ht", bufs=10))
        ytp = ctx.enter_context(tc.tile_pool(name="yt", bufs=4))
        yop = ctx.enter_context(tc.tile_pool(name="yo", bufs=3))
        ps_int = ctx.enter_context(tc.tile_pool(name="ps_int", bufs=2, space="PSUM"))
        ps_h = ctx.enter_context(tc.tile_pool(name="ps_h", bufs=2, space="PSUM"))
        ps_y = ctx.enter_context(tc.tile_pool(name="ps_y", bufs=2, space="PSUM"))
        ps_outt = ctx.enter_context(tc.tile_pool(name="ps_outt", bufs=2, space="PSUM"))

        # ---- constants / weights -------------------------------------------
        ident = const.tile([P, P], F32)
        make_identity(nc, ident[:])
        ltri = const.tile([P, P], F32)                 # ltri[k, m] = 1 if k < m
        ng.memset(ltri[:], 1.0)
        ng.affine_select(out=ltri[:], in_=ltri[:], compare_op=ALU.is_gt,
                         fill=0.0, base=0, pattern=[[1, P]], channel_multiplier=-1)

        w1s = const.tile([P, 2 * NB * DFF], F32R)
        sy.dma_start(w1s[:], w1_d[:])
        w2s = const.tile([P, (DFF // P) * NB * D], F32R)
        sy.dma_start(w2s[:], w2_d[:])
        b1s = const.tile([P, NB * (DFF // P)], F32)
        sy.dma_start(b1s[:], b1_d[:])
        b2s = const.tile([P, NB * (D // P)], F32)
        sy.dma_start(b2s[:], b2_d[:])

        # ---- init index arrays in DRAM -------------------------------------
        bt = const.tile([P, ntt], I32)
        ng.memset(bt[:], BIG)
        sarr_cov = sarr.rearrange("(p t) o -> p (t o)", p=P)
        sy.dma_start(sarr_cov[:, :], bt[:])

        # ---- phase 1: routing scan -----------------------------------------
        for _rep in range(reps):
            _phases(tc, x_d, b_d, y_d, sarr, caps, ntiles, bases, nslot, ntt,
                    scan, idxp, xgp, xtp, htp, ytp, yop,
                    ps_int, ps_h, ps_y, ps_outt,
                    ident, ltri, w1s, w2s, b1s, b2s, dbg if _rep == 0 else None)


def _phases(tc, x_d, b_d, y_d, sarr, caps, ntiles, bases, nslot, ntt,
            scan, idxp, xgp, xtp, htp, ytp, yop,
            ps_int, ps_h, ps_y, ps_outt,
            ident, ltri, w1s, w2s, b1s, b2s, dbg=None):
        nc = tc.nc
        nv = nc.vector
        ns = nc.scalar
        ng = nc.gpsimd
        sy = nc.sync
        mm_dtype = None  # unused

        b_i = scan.tile([P, JCOL], I32)
        sy.dma_start(b_i[:], b_d.rearrange("(p j) -> p j", p=P))
        b_f = scan.tile([P, JCOL], F32)
        nv.tensor_copy(b_f[:], b_i[:])

        # masks per expert: M[p, e, j] = (b == e+1)
        M = scan.tile([P, NB * JCOL], F32)
        M3 = M[:].rearrange("p (e j) -> p e j", e=NB)
        for e in range(NB):
            nv.tensor_scalar(M3[:, e, :], b_f[:], float(e + 1), None, ALU.is_equal)

        # in-row inclusive prefix sum along j (Hillis-Steele, ping-pong)
        sA = scan.tile([P, NB * JCOL], F32)
        sB = scan.tile([P, NB * JCOL], F32)
        cur, nxt = M, sA
        s = 1
        while s < JCOL:
            c3 = cur[:].rearrange("p (e j) -> p e j", e=NB)
            n3 = nxt[:].rearrange("p (e j) -> p e j", e=NB)
            nv.tensor_copy(n3[:, :, 0:s], c3[:, :, 0:s])
            nv.tensor_add(n3[:, :, s:JCOL], c3[:, :, s:JCOL], c3[:, :, 0:JCOL - s])
            cur = nxt
            nxt = sB if cur is sA else sA
            s *= 2
        incl = cur                                        # [P, NB*JCOL]

        # per-row counts and cross-partition exclusive prefix (via matmul)
        cnt = scan.tile([P, NB], F32)
        nv.tensor_reduce(cnt[:], M3[:, :, :], mybir.AxisListType.X, ALU.add)
        exr_ps = ps_h.tile([P, NB], F32, tag="hps", name="exr_ps")
        nc.tensor.matmul(exr_ps[:], ltri[:], cnt[:], start=True, stop=True)
        exr = scan.tile([P, NB], F32)
        nv.tensor_copy(exr[:], exr_ps[:])

        # candidate slot per (token, expert); select by mask; BIG for expert 0
        cand = scan.tile([P, NB * JCOL], F32)
        c3 = cand[:].rearrange("p (e j) -> p e j", e=NB)
        i3 = incl[:].rearrange("p (e j) -> p e j", e=NB)
        for e in range(NB):
            nv.tensor_scalar(c3[:, e, :], i3[:, e, :], exr[:, e:e + 1],
                             float(bases[e] - 1), ALU.add, ALU.add)
        prod = scan.tile([P, NB * JCOL], F32)
        nv.tensor_tensor(out=prod[:], in0=M[:], in1=cand[:], op=ALU.mult)
        perm_f = scan.tile([P, JCOL], F32)
        nv.tensor_reduce(perm_f[:],
                         prod[:].rearrange("p (e j) -> p j e", e=NB),
                         mybir.AxisListType.X, ALU.add)
        m0s = scan.tile([P, JCOL], F32)
        nv.tensor_scalar(m0s[:], b_f[:], 0.0, float(BIG), ALU.is_equal, ALU.mult)
        nv.tensor_add(perm_f[:], perm_f[:], m0s[:])
        perm_i = scan.tile([P, JCOL], I32)
        nv.tensor_copy(perm_i[:], perm_f[:])

        tid = scan.tile([P, JCOL], I32)
        ng.iota(tid[:], pattern=[[1, JCOL]], base=0, channel_multiplier=JCOL)
        if dbg:
            for nm, t in [("b_f", b_f), ("M", M), ("incl", incl), ("cnt", cnt),
                          ("exr", exr), ("cand", cand), ("perm_f", perm_f),
                          ("perm_i", perm_i), ("tid", tid)]:
                sy.dma_start(dbg[nm][:, :], t[:])

        for j in range(JCOL):
            ng.indirect_dma_start(
                out=sarr[:], out_offset=IndirectOffsetOnAxis(ap=perm_i[:, j:j + 1], axis=0),
                in_=tid[:, j:j + 1], in_offset=None,
                bounds_check=nslot - 1, oob_is_err=False)

        sarr_t = sarr.rearrange("(t p) o -> p (t o)", p=P)   # [P, ntt] slot t*128+p

        # ---- phase 2: per-expert FFN over supertiles -----------------------
        for e in range(NB):
            t0_e = bases[e] // P
            nt_e = ntiles[e]
            g0 = 0
            while g0 < nt_e:
                G = min(4, nt_e - g0)
                ntoks = G * P
                t0 = t0_e + g0

                idst = idxp.tile([P, 4], I32, tag="idst")
                sy.dma_start(idst[:, :G], sarr_t[:, t0:t0 + G])
                isrc = idxp.tile([P, 4], I32, tag="isrc")
                nv.tensor_scalar(isrc[:, :G], idst[:, :G], NTOK - 1, None, ALU.min)

                xg = xgp.tile([P, 4 * D], F32)
                for gi in range(G):
                    ng.indirect_dma_start(
                        out=xg[:, gi * D:(gi + 1) * D], out_offset=None,
                        in_=x_d[:], in_offset=IndirectOffsetOnAxis(ap=isrc[:, gi:gi + 1], axis=0))

                # transpose gathered [tok, d] -> xt[k][d_chunk, tok]
                xt = [xtp.tile([P, 512], F32R, tag=f"xt{k}", name=f"xt{k}") for k in range(2)]
                for k in range(2):
                    pst = ps_int.tile([P, 512], F32)
                    for gi in range(G):
                        nc.tensor.transpose(
                            out=pst[:, gi * P:(gi + 1) * P],
                            in_=xg[:, gi * D + k * P: gi * D + (k + 1) * P],
                            identity=ident[:])
                    nv.tensor_copy(xt[k][:, :ntoks], pst[:, :ntoks])

                # layer 1 + fused bias/relu -> ht[m][dff_chunk, tok]
                ht = [htp.tile([P, 512], F32R, tag="ht", name="ht") for _ in range(DFF // P)]
                for m in range(DFF // P):
                    hps = ps_h.tile([P, 512], F32)
                    for k in range(2):
                        nc.tensor.matmul(
                            hps[:, :ntoks],
                            w1s[:, (e * 2 + k) * DFF + m * P:(e * 2 + k) * DFF + (m + 1) * P],
                            xt[k][:, :ntoks],
                            start=(k == 0), stop=(k == 1))
                    ns.activation(ht[m][:, :ntoks], hps[:, :ntoks], AF.Relu,
                                  bias=b1s[:, e * (DFF // P) + m:e * (DFF // P) + m + 1],
                                  scale=1.0)

                # layer 2 + bias -> yt[c][dmodel_chunk, tok]
                yt = [ytp.tile([P, 512], F32, tag="yt", name="yt") for _ in range(D // P)]
                for c in range(D // P):
                    yps = ps_y.tile([P, 512], F32)
                    for k in range(DFF // P):
                        nc.tensor.matmul(
                            yps[:, :ntoks],
                            w2s[:, (e * (DFF // P) + k) * D + c * P:(e * (DFF // P) + k) * D + (c + 1) * P],
                            ht[k][:, :ntoks],
                            start=(k == 0), stop=(k == DFF // P - 1))
                    nv.tensor_scalar(yt[c][:, :ntoks], yps[:, :ntoks],
                                     b2s[:, e * (D // P) + c:e * (D // P) + c + 1],
                                     None, ALU.add)

                # transpose back [dmodel, tok] -> yo[tok, dmodel], 2 blocks/pack
                yo = yop.tile([P, 4 * D], F32)
                for pk in range((G + 1) // 2):
                    gis = [gi for gi in (2 * pk, 2 * pk + 1) if gi < G]
                    pso = ps_outt.tile([P, 512], F32)
                    for bi, gi in enumerate(gis):
                        for c in range(D // P):
                            nc.tensor.transpose(
                                out=pso[:, bi * D + c * P: bi * D + (c + 1) * P],
                                in_=yt[c][:, gi * P:(gi + 1) * P],
                                identity=ident[:])
                    nv.tensor_copy(yo[:, 2 * pk * D: (2 * pk + len(gis)) * D],
                                   pso[:, :len(gis) * D])

                for gi in range(G):
                    ng.indirect_dma_start(
                        out=y_d[:], out_offset=IndirectOffsetOnAxis(ap=idst[:, gi:gi + 1], axis=0),
                        in_=yo[:, gi * D:(gi + 1) * D], in_offset=None,
                        bounds_check=NTOK - 1, oob_is_err=False)
                g0 += G


def prep_inputs(x, W1, b1, W2, b2, b_seq):
    """Shard + pre-layout host-side. Returns (in_maps, caps)."""
    x = np.ascontiguousarray(np.asarray(x, dtype=np.float32))
    W1 = np.asarray(W1, dtype=np.float32)
    b1 = np.asarray(b1, dtype=np.float32)
    W2 = np.asarray(W2, dtype=np.float32)
    b2 = np.asarray(b2, dtype=np.float32)
    b_seq = np.ascontiguousarray(np.asarray(b_seq, dtype=np.int32))

    w1s = np.ascontiguousarray(
        W1.reshape(NB, 2, P, DFF).transpose(2, 0, 1, 3).reshape(P, 2 * NB * DFF))
    w2s = np.ascontiguousarray(
        W2.reshape(NB, DFF // P, P, D).transpose(2, 0, 1, 3).reshape(P, -1))
    b1s = np.ascontiguousarray(
        b1.reshape(NB, DFF // P, P).transpose(2, 0, 1).reshape(P, -1))
    b2s = np.ascontiguousarray(
        b2.reshape(NB, D // P, P).transpose(2, 0, 1).reshape(P, -1))

    bpc = B // N_CORES
    in_maps = []
    counts = np.zeros((N_CORES, NB), dtype=np.int64)
    for c in range(N_CORES):
        xc = x[c * bpc:(c + 1) * bpc].reshape(NTOK, D)
        bc = b_seq[c * bpc:(c + 1) * bpc].reshape(NTOK)
        for e in range(NB):
            counts[c, e] = int((bc == e + 1).sum())
        in_maps.append({"x": np.ascontiguousarray(xc),
                        "b": np.ascontiguousarray(bc),
                        "w1s": w1s, "w2s": w2s, "b1s": b1s, "b2s": b2s})
    caps = [max(P, int(np.ceil(counts[:, e].max() / P)) * P) for e in range(NB)]
    return in_maps, caps


def assemble(results):
    bpc = B // N_CORES
    out = np.empty((B, T, D), dtype=np.float32)
    for c in range(N_CORES):
        out[c * bpc:(c + 1) * bpc] = results[c]["y"].reshape(bpc, T, D)
    return out


def kernel(x, W1, b1, W2, b2, b_seq):
    in_maps, caps = prep_inputs(x, W1, b1, W2, b2, b_seq)
    nc = build_nc(caps)
    res = run_bass_kernel_spmd(nc, in_maps, core_ids=list(range(N_CORES)))
    return assemble(res.results)


# revision 14
# speedup vs baseline: 214.6063x; 29.4099x over previous
"""Bass/Trainium2 kernel for nn_BehaviorSpecificPFF (MoE-style routed FFN).

Reference semantics (per token t):
    e = b_seq[t]
    out[t] = 0                                   if e == 0
    out[t] = relu(x[t] @ W1[e-1] + b1[e-1]) @ W2[e-1] + b2[e-1]   otherwise

Strategy:
  - Data parallel over batch: 32 batches -> 4 per core on 8 cores.
  - Per core (8192 tokens), entirely on device:
      1. Routing scan: from b_seq compute, for every token, a unique slot in a
         per-expert bucket (matmul-based cross-partition prefix sum + shifted-add
         in-row prefix sum). Scatter token ids into two DRAM index arrays
         (gather-index, init 0; scatter-index, init BIG so padding slots are
         dropped by the bounds check).
      2. For each expert bucket, in supertiles of up to 512 slots: indirect-DMA
         gather x rows, PE-transpose to [d, tok], two matmul layers (fp32 data,
         fp32r matmul mode) with bias+relu fused on the ACT engine, PE-transpose
         back to [tok, d], indirect-DMA scatter rows to the output (padding slots
         dropped via bounds check; expert-0 rows stay zero from the zero-init
         output buffer).
  - Bucket capacities are specialized per call (max over cores, rounded to 128);
    the kernel is otherwise input-agnostic.
"""

import numpy as np

import concourse.bass as bass
import concourse.tile as tile
from concourse import bacc, mybir
from concourse.bass import IndirectOffsetOnAxis
from concourse.bass_utils import run_bass_kernel_spmd
from concourse.masks import make_identity

N_CORES = 8
B, T, D, DFF, NB = 32, 2048, 256, 1024, 4
P = 128
NTOK = B * T // N_CORES          # 8192 tokens per core
JCOL = NTOK // P                 # 64 scan columns
BIG = 100000
F32 = mybir.dt.float32
F32R = mybir.dt.float32r
I32 = mybir.dt.int32
AF = mybir.ActivationFunctionType
ALU = mybir.AluOpType


def build_nc(caps, mm_dtype=F32R, debug=False, reps=1, parts=("scan", "gather", "mm", "scatter")):
    """Build the per-core Bass program. caps: slot capacity per expert (mult of 128)."""
    ntiles = [c // P for c in caps]
    nslot = sum(caps)
    ntt = nslot // P                       # total 128-slot tiles
    bases = [sum(caps[:e]) for e in range(NB)]

    nc = bacc.Bacc("TRN2", target_bir_lowering=False, debug=False,
                   num_devices=N_CORES)
    x_d = nc.dram_tensor("x", [NTOK, D], F32, kind="ExternalInput").ap()
    b_d = nc.dram_tensor("b", [NTOK], I32, kind="ExternalInput").ap()
    w1_d = nc.dram_tensor("w1s", [P, 2 * NB * DFF], F32R, kind="ExternalInput").ap()
    w2_d = nc.dram_tensor("w2s", [P, (DFF // P) * NB * D], F32R, kind="ExternalInput").ap()
    b1_d = nc.dram_tensor("b1s", [P, NB * (DFF // P)], F32, kind="ExternalInput").ap()
    b2_d = nc.dram_tensor("b2s", [P, NB * (D // P)], F32, kind="ExternalInput").ap()
    y_d = nc.dram_tensor("y", [NTOK, D], F32, kind="ExternalOutput").ap()
    skind = "ExternalOutput" if debug else "Internal"
    dbg = {}
    if debug:
        for nm, cols in [("b_f", JCOL), ("M", NB * JCOL), ("incl", NB * JCOL),
                         ("cnt", NB), ("exr", NB), ("cand", NB * JCOL),
                         ("perm_f", JCOL)]:
            dbg[nm] = nc.dram_tensor("dbg_" + nm, [P, cols], F32, kind="ExternalOutput").ap()
        for nm, cols in [("perm_i", JCOL), ("tid", JCOL)]:
            dbg[nm] = nc.dram_tensor("dbg_" + nm, [P, cols], I32, kind="ExternalOutput").ap()
    sarr = nc.dram_tensor("sarr", [nslot, 1], I32, kind=skind).ap()

    with tile.TileContext(nc) as tc:
        _body(tc, x_d, b_d, w1_d, w2_d, b1_d, b2_d, y_d, sarr,
              caps, ntiles, bases, nslot, ntt, mm_dtype, dbg, reps, parts)
    nc.compile()
    return nc


def _body(tc, x_d, b_d, w1_d, w2_d, b1_d, b2_d, y_d, sarr,
          caps, ntiles, bases, nslot, ntt, mm_dtype, dbg=None, reps=1,
          parts=("scan", "gather", "mm", "scatter")):
    nc = tc.nc
    nv = nc.vector
    ns = nc.scalar
    ng = nc.gpsimd
    sy = nc.sync

    import contextlib
    ctx = contextlib.ExitStack()
    with ctx:
        const = ctx.enter_context(tc.tile_pool(name="const", bufs=1))
        scan = ctx.enter_context(tc.tile_pool(name="scan", bufs=1))
        idxp = ctx.enter_context(tc.tile_pool(name="idx", bufs=3))
        xgp = ctx.enter_context(tc.tile_pool(name="xg", bufs=3))
        xtp = ctx.enter_context(tc.tile_pool(name="xt", bufs=4))
        htp = ctx.enter_context(tc.tile_pool(name="ht", bufs=10))
        ytp = ctx.enter_context(tc.tile_pool(name="yt", bufs=4))
        yop = ctx.enter_context(tc.tile_pool(name="yo", bufs=3))
        ps_int = ctx.enter_context(tc.tile_pool(name="ps_int", bufs=3, space="PSUM"))
        ps_h = ctx.enter_context(tc.tile_pool(name="ps_h", bufs=3, space="PSUM"))
        ps_y = ctx.enter_context(tc.tile_pool(name="ps_y", bufs=2, space="PSUM"))
        ps_outt = ps_int

        # ---- constants / weights -------------------------------------------
        ident = const.tile([P, P], F32)
        make_identity(nc, ident[:])
        ltri = const.tile([P, P], F32)                 # ltri[k, m] = 1 if k < m
        ng.memset(ltri[:], 1.0)
        ng.affine_select(out=ltri[:], in_=ltri[:], compare_op=ALU.is_gt,
                         fill=0.0, base=0, pattern=[[1, P]], channel_multiplier=-1)

        w1s = const.tile([P, 2 * NB * DFF], F32R)
        sy.dma_start(w1s[:], w1_d[:])
        w2s = const.tile([P, (DFF // P) * NB * D], F32R)
        sy.dma_start(w2s[:], w2_d[:])
        b1s = const.tile([P, NB * (DFF // P)], F32)
        sy.dma_start(b1s[:], b1_d[:])
        b2s = const.tile([P, NB * (D // P)], F32)
        sy.dma_start(b2s[:], b2_d[:])

        # ---- init index arrays in DRAM -------------------------------------
        bt = const.tile([P, ntt], I32)
        ng.memset(bt[:], BIG)
        sarr_cov = sarr.rearrange("(p t) o -> p (t o)", p=P)
        sy.dma_start(sarr_cov[:, :], bt[:])

        # ---- phase 1: routing scan -----------------------------------------
        for _rep in range(reps):
            _phases(tc, x_d, b_d, y_d, sarr, caps, ntiles, bases, nslot, ntt,
                    scan, idxp, xgp, xtp, htp, ytp, yop,
                    ps_int, ps_h, ps_y, ps_outt,
                    ident, ltri, w1s, w2s, b1s, b2s, dbg if _rep == 0 else None,
                    parts)


def _phases(tc, x_d, b_d, y_d, sarr, caps, ntiles, bases, nslot, ntt,
            scan, idxp, xgp, xtp, htp, ytp, yop,
            ps_int, ps_h, ps_y, ps_outt,
            ident, ltri, w1s, w2s, b1s, b2s, dbg=None,
            parts=("scan", "gather", "mm", "scatter")):
        nc = tc.nc
        nv = nc.vector
        ns = nc.scalar
        ng = nc.gpsimd
        sy = nc.sync
        mm_dtype = None  # unused

        if "scan" in parts:
            _scan_phase(tc, b_d, sarr, bases, nslot, scan, ps_h, ltri, dbg)
        _ffn_phase(tc, x_d, y_d, sarr, caps, ntiles, bases, nslot, ntt,
                   idxp, xgp, xtp, htp, ytp, yop, ps_int, ps_h, ps_y, ps_outt,
                   ident, w1s, w2s, b1s, b2s, parts)


def _scan_phase(tc, b_d, sarr, bases, nslot, scan, ps_h, ltri, dbg=None):
        nc = tc.nc
        nv = nc.vector
        ng = nc.gpsimd
        sy = nc.sync

        b_i = scan.tile([P, JCOL], I32)
        sy.dma_start(b_i[:], b_d.rearrange("(p j) -> p j", p=P))
        b_f = scan.tile([P, JCOL], F32)
        nv.tensor_copy(b_f[:], b_i[:])

        # masks per expert: M[p, e, j] = (b == e+1)
        M = scan.tile([P, NB * JCOL], F32)
        M3 = M[:].rearrange("p (e j) -> p e j", e=NB)
        for e in range(NB):
            nv.tensor_scalar(M3[:, e, :], b_f[:], float(e + 1), None, ALU.is_equal)

        # in-row inclusive prefix sum along j (Hillis-Steele, ping-pong)
        sA = scan.tile([P, NB * JCOL], F32)
        sB = scan.tile([P, NB * JCOL], F32)
        cur, nxt = M, sA
        s = 1
        while s < JCOL:
            c3 = cur[:].rearrange("p (e j) -> p e j", e=NB)
            n3 = nxt[:].rearrange("p (e j) -> p e j", e=NB)
            nv.tensor_copy(n3[:, :, 0:s], c3[:, :, 0:s])
            nv.tensor_add(n3[:, :, s:JCOL], c3[:, :, s:JCOL], c3[:, :, 0:JCOL - s])
            cur = nxt
            nxt = sB if cur is sA else sA
            s *= 2
        incl = cur                                        # [P, NB*JCOL]

        # per-row counts and cross-partition exclusive prefix (via matmul)
        cnt = scan.tile([P, NB], F32)
        nv.tensor_reduce(cnt[:], M3[:, :, :], mybir.AxisListType.X, ALU.add)
        exr_ps = ps_h.tile([P, NB], F32, tag="hps", name="exr_ps")
        nc.tensor.matmul(exr_ps[:], ltri[:], cnt[:], start=True, stop=True)
        exr = scan.tile([P, NB], F32)
        nv.tensor_copy(exr[:], exr_ps[:])

        # candidate slot per (token, expert); select by mask; BIG for expert 0
        cand = scan.tile([P, NB * JCOL], F32)
        c3 = cand[:].rearrange("p (e j) -> p e j", e=NB)
        i3 = incl[:].rearrange("p (e j) -> p e j", e=NB)
        for e in range(NB):
            nv.tensor_scalar(c3[:, e, :], i3[:, e, :], exr[:, e:e + 1],
                             float(bases[e] - 1), ALU.add, ALU.add)
        prod = scan.tile([P, NB * JCOL], F32)
        nv.tensor_tensor(out=prod[:], in0=M[:], in1=cand[:], op=ALU.mult)
        perm_f = scan.tile([P, JCOL], F32)
        nv.tensor_reduce(perm_f[:],
                         prod[:].rearrange("p (e j) -> p j e", e=NB),
                         mybir.AxisListType.X, ALU.add)
        m0s = scan.tile([P, JCOL], F32)
        nv.tensor_scalar(m0s[:], b_f[:], 0.0, float(BIG), ALU.is_equal, ALU.mult)
        nv.tensor_add(perm_f[:], perm_f[:], m0s[:])
        perm_i = scan.tile([P, JCOL], I32)
        nv.tensor_copy(perm_i[:], perm_f[:])

        tid = scan.tile([P, JCOL], I32)
        ng.iota(tid[:], pattern=[[1, JCOL]], base=0, channel_multiplier=JCOL)
        if dbg:
            for nm, t in [("b_f", b_f), ("M", M), ("incl", incl), ("cnt", cnt),
                          ("exr", exr), ("cand", cand), ("perm_f", perm_f),
                          ("perm_i", perm_i), ("tid", tid)]:
                sy.dma_start(dbg[nm][:, :], t[:])

        for j in range(JCOL):
            ng.indirect_dma_start(
                out=sarr[:], out_offset=IndirectOffsetOnAxis(ap=perm_i[:, j:j + 1], axis=0),
                in_=tid[:, j:j + 1], in_offset=None,
                bounds_check=nslot - 1, oob_is_err=False)


def _ffn_phase(tc, x_d, y_d, sarr, caps, ntiles, bases, nslot, ntt,
               idxp, xgp, xtp, htp, ytp, yop, ps_int, ps_h, ps_y, ps_outt,
               ident, w1s, w2s, b1s, b2s, parts):
    nc = tc.nc
    nv = nc.vector
    ns = nc.scalar
    ng = nc.gpsimd
    sy = nc.sync
    sarr_t = sarr.rearrange("(t p) o -> p (t o)", p=P)

    for e in range(NB):
        t0_e = bases[e] // P
        nt_e = ntiles[e]
        g0 = 0
        while g0 < nt_e:
            G = min(4, nt_e - g0)
            ntoks = G * P
            t0 = t0_e + g0

            idst = idxp.tile([P, 4], I32, tag="idst")
            sy.dma_start(idst[:, :G], sarr_t[:, t0:t0 + G])
            isrc = idxp.tile([P, 4], I32, tag="isrc")
            nv.tensor_scalar(isrc[:, :G], idst[:, :G], NTOK - 1, None, ALU.min)

            xg = xgp.tile([P, 4 * D], F32)
            if "mm" in parts and "gather" not in parts:
                nv.memset(xg[:], 0.0)
            if "gather" in parts:
                for gi in range(G):
                    ng.indirect_dma_start(
                        out=xg[:, gi * D:(gi + 1) * D], out_offset=None,
                        in_=x_d[:],
                        in_offset=IndirectOffsetOnAxis(ap=isrc[:, gi:gi + 1], axis=0))

            yo = yop.tile([P, 4 * D], F32)
            if "scatter" in parts and "mm" not in parts:
                nv.memset(yo[:], 0.0)
            if "mm" in parts:
                # transpose gathered [tok, d] -> xt[k][d_chunk, tok]
                xt = [xtp.tile([P, 512], F32R, tag=f"xt{k}", name=f"xt{k}")
                      for k in range(2)]
                for k in range(2):
                    pst = ps_int.tile([P, 512], F32, tag="pst", name="pst")
                    for gi in range(G):
                        nc.tensor.transpose(
                            out=pst[:, gi * P:(gi + 1) * P],
                            in_=xg[:, gi * D + k * P: gi * D + (k + 1) * P],
                            identity=ident[:])
                    nv.tensor_copy(xt[k][:, :ntoks], pst[:, :ntoks])

                # layer 1 + fused bias/relu -> ht[m][dff_chunk, tok]
                ht = [htp.tile([P, 512], F32R, tag="ht", name="ht")
                      for _ in range(DFF // P)]
                for m in range(DFF // P):
                    hps = ps_h.tile([P, 512], F32)
                    for k in range(2):
                        nc.tensor.matmul(
                            hps[:, :ntoks],
                            w1s[:, (e * 2 + k) * DFF + m * P:(e * 2 + k) * DFF + (m + 1) * P],
                            xt[k][:, :ntoks],
                            start=(k == 0), stop=(k == 1))
                    if m % 2 == 0:
                        ns.activation(ht[m][:, :ntoks], hps[:, :ntoks], AF.Relu,
                                      bias=b1s[:, e * (DFF // P) + m:e * (DFF // P) + m + 1],
                                      scale=1.0)
                    else:
                        nv.tensor_scalar(ht[m][:, :ntoks], hps[:, :ntoks],
                                         b1s[:, e * (DFF // P) + m:e * (DFF // P) + m + 1],
                                         0.0, ALU.add, ALU.max)

                # layer 2 + bias -> yt[c][dmodel_chunk, tok]
                yt = [ytp.tile([P, 512], F32, tag="yt", name="yt")
                      for _ in range(D // P)]
                for c in range(D // P):
                    yps = ps_y.tile([P, 512], F32)
                    for k in range(DFF // P):
                        nc.tensor.matmul(
                            yps[:, :ntoks],
                            w2s[:, (e * (DFF // P) + k) * D + c * P:(e * (DFF // P) + k) * D + (c + 1) * P],
                            ht[k][:, :ntoks],
                            start=(k == 0), stop=(k == DFF // P - 1))
                    nv.tensor_scalar(yt[c][:, :ntoks], yps[:, :ntoks],
                                     b2s[:, e * (D // P) + c:e * (D // P) + c + 1],
                                     None, ALU.add)

                # transpose back [dmodel, tok] -> yo[tok, dmodel], 2 blocks/pack
                for pk in range((G + 1) // 2):
                    gis = [gi for gi in (2 * pk, 2 * pk + 1) if gi < G]
                    pso = ps_outt.tile([P, 512], F32, tag="pst", name="pso")
                    for bi, gi in enumerate(gis):
                        for c in range(D // P):
                            nc.tensor.transpose(
                                out=pso[:, bi * D + c * P: bi * D + (c + 1) * P],
                                in_=yt[c][:, gi * P:(gi + 1) * P],
                                identity=ident[:])
                    nv.tensor_copy(yo[:, 2 * pk * D: (2 * pk + len(gis)) * D],
                                   pso[:, :len(gis) * D])

            if "scatter" in parts:
                sidx = idst if "scan" in parts else isrc
                for gi in range(G):
                    ng.indirect_dma_start(
                        out=y_d[:],
                        out_offset=IndirectOffsetOnAxis(ap=sidx[:, gi:gi + 1], axis=0),
                        in_=yo[:, gi * D:(gi + 1) * D], in_offset=None,
                        bounds_check=NTOK - 1, oob_is_err=False)
            g0 += G


def prep_inputs(x, W1, b1, W2, b2, b_seq):
    """Shard + pre-layout host-side. Returns (in_maps, caps)."""
    x = np.ascontiguousarray(np.asarray(x, dtype=np.float32))
    W1 = np.asarray(W1, dtype=np.float32)
    b1 = np.asarray(b1, dtype=np.float32)
    W2 = np.asarray(W2, dtype=np.float32)
    b2 = np.asarray(b2, dtype=np.float32)
    b_seq = np.ascontiguousarray(np.asarray(b_seq, dtype=np.int32))

    w1s = np.ascontiguousarray(
        W1.reshape(NB, 2, P, DFF).transpose(2, 0, 1, 3).reshape(P, 2 * NB * DFF))
    w2s = np.ascontiguousarray(
        W2.reshape(NB, DFF // P, P, D).transpose(2, 0, 1, 3).reshape(P, -1))
    b1s = np.ascontiguousarray(
        b1.reshape(NB, DFF // P, P).transpose(2, 0, 1).reshape(P, -1))
    b2s = np.ascontiguousarray(
        b2.reshape(NB, D // P, P).transpose(2, 0, 1).reshape(P, -1))

    bpc = B // N_CORES
    in_maps = []
    counts = np.zeros((N_CORES, NB), dtype=np.int64)
    for c in range(N_CORES):
        xc = x[c * bpc:(c + 1) * bpc].reshape(NTOK, D)
        bc = b_seq[c * bpc:(c + 1) * bpc].reshape(NTOK)
        for e in range(NB):
            counts[c, e] = int((bc == e + 1).sum())
        in_maps.append({"x": np.ascontiguousarray(xc),
                        "b": np.ascontiguousarray(bc),
                        "w1s": w1s, "w2s": w2s, "b1s": b1s, "b2s": b2s})
    caps = [max(P, int(np.ceil(counts[:, e].max() / P)) * P) for e in range(NB)]
    return in_maps, caps


def assemble(results):
    bpc = B // N_CORES
    out = np.empty((B, T, D), dtype=np.float32)
    for c in range(N_CORES):
        out[c * bpc:(c + 1) * bpc] = results[c]["y"].reshape(bpc, T, D)
    return out


def kernel(x, W1, b1, W2, b2, b_seq):
    in_maps, caps = prep_inputs(x, W1, b1, W2, b2, b_seq)
    nc = build_nc(caps)
    res = run_bass_kernel_spmd(nc, in_maps, core_ids=list(range(N_CORES)))
    return assemble(res.results)
